# revision 12
# baseline (speedup 1.0000x reference)
"""Trainium2 Bass kernel for NoSharingGraphConv.

out[b,w,m] = sum_{h,n} x[b,h,n] * adj[h,w] * W[h,w,n,m] + bias[m]
  B=4096, N=17 (graph nodes), FIN=FOUT=256.

Strategy (8 NeuronCores, data-parallel over batch, 512 rows/core):
  - Host pre-transposes each x shard to xT [17*256, 512]; xT resides in SBUF
    for the whole kernel (the matmul moving operand, free dim = batch 512).
  - W streams through SBUF one w-slab at a time ([17,2,256] blocks of 128
    contraction rows); each slab is scaled in-place by adj[:,w] on the DVE
    (adj is broadcast across partitions once via a K=1 ones-matmul).
  - Per (w, m-tile): 34 accumulating float32r matmuls [128x128]x[128x512]
    into one PSUM bank; ACT evacuates PSUM with the per-partition bias add.
  - Device writes out_t [17,256,512] (w, m, b); host permutes to (b, w, m).
"""

import sys

if "/opt/trn_rl_repo" not in sys.path:
    sys.path.insert(0, "/opt/trn_rl_repo")

import numpy as np

B, N, FIN, FOUT = 4096, 17, 256, 256
NC = 8
BS = B // NC  # 512 batch rows per core
KCH = N * FIN // 128  # 34 contraction chunks of 128
MT = FOUT // 128  # 2 output-feature tiles of 128

_CACHE = {}


def _build_module():
    import concourse.mybir as mybir
    import concourse.tile as tile
    from concourse import bacc

    f32 = mybir.dt.float32
    f32r = mybir.dt.float32r

    nc = bacc.Bacc("TRN2", target_bir_lowering=False)

    # float32r = 4-byte fp32 reinterpreted as PE "fp32 reduced" (FP22 in the
    # array, fp32 accumulate) — runs the matmul at full 1 cycle/row rate.
    xt_d = nc.dram_tensor("xt", [N * FIN, BS], f32r, kind="ExternalInput")
    w_d = nc.dram_tensor("w", [N, N, FIN, FOUT], f32r, kind="ExternalInput")
    adj_d = nc.dram_tensor("adj", [N, N], f32, kind="ExternalInput")
    b_d = nc.dram_tensor("b", [FOUT], f32, kind="ExternalInput")
    o_d = nc.dram_tensor("out_t", [N, FOUT, BS], f32, kind="ExternalOutput")

    with tile.TileContext(nc) as tc:
        with (
            tc.tile_pool(name="const", bufs=1) as const,
            tc.tile_pool(name="wslab", bufs=3) as wpool,
            tc.tile_pool(name="obuf", bufs=4) as opool,
            tc.tile_pool(name="psum", bufs=4, space="PSUM") as psum,
        ):
            # adj, transposed to (w, h) order, on partition 0, then
            # replicated across all 128 partitions (gpsimd custom inst)
            adj_row = const.tile([1, N, N], f32)
            nc.sync.dma_start(adj_row[:], adj_d[:].rearrange("h w -> w h")[None])
            adj_sb = const.tile([128, N, N], f32)  # [p][w][h]
            nc.gpsimd.partition_broadcast(adj_sb[:], adj_row[:])

            # bias striped to [128, MT]: bias_sb[p, mt] = b[mt*128 + p]
            bias_sb = const.tile([128, MT], f32)
            nc.sync.dma_start(bias_sb[:], b_d[:].rearrange("(mt p) -> p mt", p=128))

            # resident x^T: xt_sb[p, c, b] = x_flat[b, c*128+p]
            # (ACT-ring DMA so w-slab loads on the SP ring run in parallel)
            xt_sb = const.tile([128, KCH, BS], f32r)
            nc.scalar.dma_start(xt_sb[:], xt_d[:].rearrange("(c p) b -> p c b", p=128))

            for w in range(N):
                # slab[p, h, kc*256+m] = W[h, w, kc*128+p, m]
                wt = wpool.tile([128, N, 2 * FOUT], f32r, tag="wslab")
                for kc in range(2):
                    nc.sync.dma_start(
                        wt[:, :, kc * FOUT : (kc + 1) * FOUT],
                        w_d[:, w][:, kc * 128 : (kc + 1) * 128, :].rearrange(
                            "h p m -> p h m"
                        ),
                    )
                # scale by adj[h, w], broadcast over (kc, m)
                nc.vector.tensor_tensor(
                    wt[:],
                    wt[:],
                    adj_sb[:, w][:, :, None].to_broadcast([128, N, 2 * FOUT]),
                    mybir.AluOpType.mult,
                )
                for mt in range(MT):
                    ps = psum.tile([128, BS], mybir.dt.float32, tag="ps")
                    for c in range(KCH):
                        h, kc = divmod(c, 2)
                        off = kc * FOUT + mt * 128
                        nc.tensor.matmul(
                            ps[:],
                            lhsT=wt[:, h, off : off + 128],
                            rhs=xt_sb[:, c, :],
                            start=(c == 0),
                            stop=(c == KCH - 1),
                        )
                    ot = opool.tile([128, BS], f32, tag="ot")
                    nc.scalar.activation(
                        ot[:],
                        ps[:],
                        mybir.ActivationFunctionType.Identity,
                        bias=bias_sb[:, mt : mt + 1],
                    )
                    nc.scalar.dma_start(o_d[w, mt * 128 : (mt + 1) * 128, :], ot[:])

    nc.compile()
    return nc


def _get_module():
    if "nc" not in _CACHE:
        _CACHE["nc"] = _build_module()
    return _CACHE["nc"]


def kernel(x, adj, W, b, _trace=False):
    from concourse.bass_utils import run_bass_kernel_spmd

    x = np.ascontiguousarray(np.asarray(x, dtype=np.float32))
    adj = np.ascontiguousarray(np.asarray(adj, dtype=np.float32))
    W = np.ascontiguousarray(np.asarray(W, dtype=np.float32))
    b = np.ascontiguousarray(np.asarray(b, dtype=np.float32))

    nc = _get_module()

    in_maps = []
    for c in range(NC):
        xs = x[c * BS : (c + 1) * BS].reshape(BS, N * FIN)
        xt = np.ascontiguousarray(xs.T)  # [4352, 512]
        in_maps.append({"xt": xt, "w": W, "adj": adj, "b": b})

    res = run_bass_kernel_spmd(nc, in_maps, list(range(NC)), trace=_trace)
    _CACHE["last_result"] = res

    out = np.empty((B, N, FOUT), dtype=np.float32)
    for c in range(NC):
        ot = res.results[c]["out_t"]  # [17, 256, 512]
        out[c * BS : (c + 1) * BS] = ot.transpose(2, 0, 1)
    return out


# revision 15
# speedup vs baseline: 1.0051x; 1.0051x over previous
"""Trainium2 Bass kernel for NoSharingGraphConv.

out[b,w,m] = sum_{h,n} x[b,h,n] * adj[h,w] * W[h,w,n,m] + bias[m]
  B=4096, N=17 (graph nodes), FIN=FOUT=256.

Strategy (8 NeuronCores, data-parallel over batch, 512 rows/core):
  - Host pre-transposes each x shard to xT [17*256, 512]; xT resides in SBUF
    for the whole kernel (the matmul moving operand, free dim = batch 512).
  - W streams through SBUF one w-slab at a time ([17,2,256] blocks of 128
    contraction rows); each slab is scaled in-place by adj[:,w] on the DVE
    (adj is broadcast across partitions once via a K=1 ones-matmul).
  - Per (w, m-tile): 34 accumulating float32r matmuls [128x128]x[128x512]
    into one PSUM bank; ACT evacuates PSUM with the per-partition bias add.
  - Device writes out_t [17,256,512] (w, m, b); host permutes to (b, w, m).
"""

import sys

if "/opt/trn_rl_repo" not in sys.path:
    sys.path.insert(0, "/opt/trn_rl_repo")

import numpy as np

B, N, FIN, FOUT = 4096, 17, 256, 256
NC = 8
BS = B // NC  # 512 batch rows per core
KCH = N * FIN // 128  # 34 contraction chunks of 128
MT = FOUT // 128  # 2 output-feature tiles of 128

_CACHE = {}


def _build_module():
    import concourse.mybir as mybir
    import concourse.tile as tile
    from concourse import bacc

    f32 = mybir.dt.float32
    f32r = mybir.dt.float32r

    nc = bacc.Bacc("TRN2", target_bir_lowering=False)

    # float32r = 4-byte fp32 reinterpreted as PE "fp32 reduced" (FP22 in the
    # array, fp32 accumulate) — runs the matmul at full 1 cycle/row rate.
    xt_d = nc.dram_tensor("xt", [N * FIN, BS], f32r, kind="ExternalInput")
    w_d = nc.dram_tensor("w", [N, N, FIN, FOUT], f32r, kind="ExternalInput")
    adj_d = nc.dram_tensor("adj", [N, N], f32, kind="ExternalInput")
    b_d = nc.dram_tensor("b", [FOUT], f32, kind="ExternalInput")
    o_d = nc.dram_tensor("out_t", [N, FOUT, BS], f32, kind="ExternalOutput")

    with tile.TileContext(nc) as tc:
        with (
            tc.tile_pool(name="const", bufs=1) as const,
            tc.tile_pool(name="wslab", bufs=3) as wpool,
            tc.tile_pool(name="obuf", bufs=4) as opool,
            tc.tile_pool(name="psum", bufs=4, space="PSUM") as psum,
        ):
            # adj, transposed to (w, h) order, on partition 0, then
            # replicated across all 128 partitions (gpsimd custom inst)
            adj_row = const.tile([1, N, N], f32)
            nc.sync.dma_start(adj_row[:], adj_d[:].rearrange("h w -> w h")[None])
            adj_sb = const.tile([128, N, N], f32)  # [p][w][h]
            nc.gpsimd.partition_broadcast(adj_sb[:], adj_row[:])

            # bias striped to [128, MT]: bias_sb[p, mt] = b[mt*128 + p]
            bias_sb = const.tile([128, MT], f32)
            nc.sync.dma_start(bias_sb[:], b_d[:].rearrange("(mt p) -> p mt", p=128))

            # resident x^T, host-permuted so chunk c=(h,kc) row p holds
            # x[b, h, 2p+kc] (matches the interleaved W slab layout).
            # Split into 4 DMAs so the first matmuls start early.
            # (ACT-ring DMAs so w-slab loads on the SP ring run in parallel)
            xt_sb = const.tile([128, KCH, BS], f32r)
            xt_src = xt_d[:].rearrange("(c p) b -> p c b", p=128)
            for c0, c1 in ((0, 9), (9, 18), (18, 26), (26, KCH)):
                nc.scalar.dma_start(xt_sb[:, c0:c1, :], xt_src[:, c0:c1, :])

            for w in range(N):
                # slab[p, h, kc*256+m] = W[h, w, 2p+kc, m] — partition line
                # reads two adjacent 1KB m-rows (2KB contiguous DMA runs)
                wt = wpool.tile([128, N, 2 * FOUT], f32r, tag="wslab")
                nc.sync.dma_start(
                    wt[:],
                    w_d[:, w].rearrange("h (p two) m -> p h (two m)", two=2),
                )
                # scale by adj[h, w]; per-h tensor_scalar runs in DVE 2x mode
                for h in range(N):
                    nc.vector.tensor_scalar_mul(
                        wt[:, h, :], wt[:, h, :], adj_sb[:, w, h : h + 1]
                    )
                for mt in range(MT):
                    ps = psum.tile([128, BS], mybir.dt.float32, tag="ps")
                    for c in range(KCH):
                        h, kc = divmod(c, 2)
                        off = kc * FOUT + mt * 128
                        nc.tensor.matmul(
                            ps[:],
                            lhsT=wt[:, h, off : off + 128],
                            rhs=xt_sb[:, c, :],
                            start=(c == 0),
                            stop=(c == KCH - 1),
                        )
                    ot = opool.tile([128, BS], f32, tag="ot")
                    nc.scalar.activation(
                        ot[:],
                        ps[:],
                        mybir.ActivationFunctionType.Identity,
                        bias=bias_sb[:, mt : mt + 1],
                    )
                    nc.scalar.dma_start(o_d[w, mt * 128 : (mt + 1) * 128, :], ot[:])

    nc.compile()
    return nc


def _get_module():
    if "nc" not in _CACHE:
        _CACHE["nc"] = _build_module()
    return _CACHE["nc"]


def kernel(x, adj, W, b, _trace=False):
    from concourse.bass_utils import run_bass_kernel_spmd

    x = np.ascontiguousarray(np.asarray(x, dtype=np.float32))
    adj = np.ascontiguousarray(np.asarray(adj, dtype=np.float32))
    W = np.ascontiguousarray(np.asarray(W, dtype=np.float32))
    b = np.ascontiguousarray(np.asarray(b, dtype=np.float32))

    nc = _get_module()

    in_maps = []
    for c in range(NC):
        xs = x[c * BS : (c + 1) * BS]  # [BS, N, FIN]
        # xt[(h*2+kc)*128 + p, b] = x[b, h, 2p+kc] — interleaved-n layout
        # matching the device W slab (n = 2p + kc)
        xr = xs.reshape(BS, N, FIN // 2, 2)
        xt = np.ascontiguousarray(
            xr.transpose(1, 3, 2, 0).reshape(N * FIN, BS)
        )  # [4352, 512]
        in_maps.append({"xt": xt, "w": W, "adj": adj, "b": b})

    res = run_bass_kernel_spmd(nc, in_maps, list(range(NC)), trace=_trace)
    _CACHE["last_result"] = res

    out = np.empty((B, N, FOUT), dtype=np.float32)
    for c in range(NC):
        ot = res.results[c]["out_t"]  # [17, 256, 512]
        out[c * BS : (c + 1) * BS] = ot.transpose(2, 0, 1)
    return out


# revision 16
# speedup vs baseline: 1.0120x; 1.0069x over previous
"""Trainium2 Bass kernel for NoSharingGraphConv.

out[b,w,m] = sum_{h,n} x[b,h,n] * adj[h,w] * W[h,w,n,m] + bias[m]
  B=4096, N=17 (graph nodes), FIN=FOUT=256.

Sharding (8 NeuronCores): 4 batch groups x 2 out-feature halves.
Core c handles batch rows [bg*1024, (bg+1)*1024) and out features
[mh*128, (mh+1)*128), bg = c>>1, mh = c&1. This halves the per-core W
stream (37.9MB) vs pure batch-parallel while keeping the PE work
perfectly balanced (1156 matmuls of [128x128]x[128x512] per core).

Device kernel (per core):
  - x^T shard resident in SBUF [128, 34, 1024] (host-transposed, n
    interleaved as n = 2p+kc so it matches the W slab layout).
  - W streamed one w-slab at a time; host pre-swizzles W into the exact
    slab layout [w, p, h, kc, m'] so each slab DMA is one fully
    contiguous 2.2MB read (17.4KB per partition line).
  - Slab scaled in-place by adj[:,w] on the DVE (per-h tensor_scalar,
    2x mode); adj is broadcast across partitions once via gpsimd.
  - Per (w, batch-half): 34 accumulating float32r matmuls into one PSUM
    bank; ACT evacuates with the per-partition bias add (fp32).
  - Device writes out_t [17, 128, 1024] (w, m', b); host permutes back.
"""

import sys

if "/opt/trn_rl_repo" not in sys.path:
    sys.path.insert(0, "/opt/trn_rl_repo")

import numpy as np

B, N, FIN, FOUT = 4096, 17, 256, 256
NC = 8
NBG = 4  # batch groups
BS = B // NBG  # 1024 batch rows per core
MH = FOUT // 2  # 128 out features per core
KCH = N * FIN // 128  # 34 contraction chunks of 128
NBH = BS // 512  # 2 batch halves (matmul free dim 512)

_CACHE = {}


def _build_module():
    import concourse.mybir as mybir
    import concourse.tile as tile
    from concourse import bacc

    f32 = mybir.dt.float32
    f32r = mybir.dt.float32r

    nc = bacc.Bacc("TRN2", target_bir_lowering=False)

    xt_d = nc.dram_tensor("xt", [N * FIN, BS], f32r, kind="ExternalInput")
    # host-swizzled: w_sw[w, p, h, kc, m'] = adj-unscaled W[h, w, 2p+kc, mh*128+m']
    w_d = nc.dram_tensor("w_sw", [N, 128, N, 2, MH], f32r, kind="ExternalInput")
    adj_d = nc.dram_tensor("adj", [N, N], f32, kind="ExternalInput")
    b_d = nc.dram_tensor("b", [MH], f32, kind="ExternalInput")
    o_d = nc.dram_tensor("out_t", [N, MH, BS], f32, kind="ExternalOutput")

    with tile.TileContext(nc) as tc:
        with (
            tc.tile_pool(name="const", bufs=1) as const,
            tc.tile_pool(name="wslab", bufs=2) as wpool,
            tc.tile_pool(name="obuf", bufs=4) as opool,
            tc.tile_pool(name="psum", bufs=4, space="PSUM") as psum,
        ):
            # adj, transposed to (w, h) order, on partition 0, then
            # replicated across all 128 partitions (gpsimd custom inst)
            adj_row = const.tile([1, N, N], f32)
            nc.sync.dma_start(adj_row[:], adj_d[:].rearrange("h w -> w h")[None])
            adj_sb = const.tile([128, N, N], f32)  # [p][w][h]
            nc.gpsimd.partition_broadcast(adj_sb[:], adj_row[:])

            # bias half on partitions: bias_sb[p, 0] = b[mh*128 + p]
            bias_sb = const.tile([128, 1], f32)
            nc.sync.dma_start(bias_sb[:], b_d[:][:, None])

            # resident x^T, host-permuted: chunk c=(h,kc) row p holds
            # x[b, h, 2p+kc]. Split DMAs: batch half-major so the first
            # psum group's operands land first. (ACT ring, so w-slab
            # loads on the SP ring run in parallel.)
            xt_sb = const.tile([128, KCH, BS], f32r)
            xt_src = xt_d[:].rearrange("(c p) b -> p c b", p=128)
            for b0, b1 in ((0, 512), (512, BS)):
                for c0, c1 in ((0, 9), (9, 18), (18, 26), (26, KCH)):
                    nc.scalar.dma_start(
                        xt_sb[:, c0:c1, b0:b1], xt_src[:, c0:c1, b0:b1]
                    )

            for w in range(N):
                # one fully-contiguous 2.2MB slab read per w
                wt = wpool.tile([128, N, 2, MH], f32r, tag="wslab")
                nc.sync.dma_start(
                    wt[:].rearrange("p h kc m -> p (h kc m)"),
                    w_d[w].rearrange("p h kc m -> p (h kc m)"),
                )
                # scale by adj[h, w]; per-h tensor_scalar runs in DVE 2x mode
                for h in range(N):
                    nc.vector.tensor_scalar_mul(
                        wt[:, h].rearrange("p kc m -> p (kc m)"),
                        wt[:, h].rearrange("p kc m -> p (kc m)"),
                        adj_sb[:, w, h : h + 1],
                    )
                for bh in range(NBH):
                    ps = psum.tile([128, 512], mybir.dt.float32, tag="ps")
                    for c in range(KCH):
                        h, kc = divmod(c, 2)
                        nc.tensor.matmul(
                            ps[:],
                            lhsT=wt[:, h, kc, :],
                            rhs=xt_sb[:, c, bh * 512 : (bh + 1) * 512],
                            start=(c == 0),
                            stop=(c == KCH - 1),
                        )
                    ot = opool.tile([128, 512], f32, tag="ot")
                    nc.scalar.activation(
                        ot[:],
                        ps[:],
                        mybir.ActivationFunctionType.Identity,
                        bias=bias_sb[:, 0:1],
                    )
                    nc.scalar.dma_start(
                        o_d[w, :, bh * 512 : (bh + 1) * 512], ot[:]
                    )

    nc.compile()
    return nc


def _get_module():
    if "nc" not in _CACHE:
        _CACHE["nc"] = _build_module()
    return _CACHE["nc"]


def kernel(x, adj, W, b, _trace=False):
    from concourse.bass_utils import run_bass_kernel_spmd

    x = np.ascontiguousarray(np.asarray(x, dtype=np.float32))
    adj = np.ascontiguousarray(np.asarray(adj, dtype=np.float32))
    W = np.ascontiguousarray(np.asarray(W, dtype=np.float32))
    b = np.ascontiguousarray(np.asarray(b, dtype=np.float32))

    nc = _get_module()

    # W pre-swizzled per m-half: [w, p, h, kc, m'] = W[h, w, 2p+kc, mh*128+m']
    w_sw = []
    for mh in range(2):
        wh = W[:, :, :, mh * MH : (mh + 1) * MH]  # [h, w, n, m']
        wr = wh.reshape(N, N, FIN // 2, 2, MH)  # (h, w, p, kc, m')
        w_sw.append(np.ascontiguousarray(wr.transpose(1, 2, 0, 3, 4)))

    xt_by_bg = []
    for bg in range(NBG):
        xs = x[bg * BS : (bg + 1) * BS]  # [BS, N, FIN]
        # xt[(h*2+kc)*128 + p, b] = x[b, h, 2p+kc]
        xr = xs.reshape(BS, N, FIN // 2, 2)
        xt_by_bg.append(
            np.ascontiguousarray(xr.transpose(1, 3, 2, 0).reshape(N * FIN, BS))
        )

    in_maps = []
    for c in range(NC):
        bg, mh = divmod(c, 2)
        in_maps.append(
            {
                "xt": xt_by_bg[bg],
                "w_sw": w_sw[mh],
                "adj": adj,
                "b": b[mh * MH : (mh + 1) * MH].copy(),
            }
        )

    res = run_bass_kernel_spmd(nc, in_maps, list(range(NC)), trace=_trace)
    _CACHE["last_result"] = res

    out = np.empty((B, N, FOUT), dtype=np.float32)
    for c in range(NC):
        bg, mh = divmod(c, 2)
        ot = res.results[c]["out_t"]  # [17, 128, 1024] = (w, m', b)
        out[bg * BS : (bg + 1) * BS, :, mh * MH : (mh + 1) * MH] = ot.transpose(
            2, 0, 1
        )
    return out
